# revision 10
# baseline (speedup 1.0000x reference)
"""Causal single-head attention on 8 Trainium2 NeuronCores.

Problem: x [4, 2048, 1024] f32; Wq/Wk/Wv [1024, 1024] f32.
  q,k,v = x@W*; out = softmax(causal(q k^T / sqrt(d))) @ v.

Sharding: 8 cores = 4 batches x 2 query-halves. Causal load balancing via
"fold" assignment of 512-query blocks: core (b, 0) takes query blocks
{3, 0} of its batch, core (b, 1) takes {2, 1}. Each core owns two
512-query "supers" whose key-prefix lengths are padded to the
compile-time slot shape (16, 8) x 128-key tiles; the pad region is
neutralized by an additive -60000 mask (host-built, tiny) so a single
program serves all cores (SPMD).

The k/v projections are split across each core pair: every core projects
only its half of the batch tokens ([0:1024] on even ranks, [1024:2048]
on odd ranks) and the halves are exchanged with a pair-wise AllGather
through DRAM bounce buffers. The gather output is in rank order == global
token order, so the program stays uniform across cores.

All matmul operands are fp16 (1 PE cycle/row like bf16 but 4x finer
mantissa; fp32 runs 4x slower and fp32r measures 2 cycles/row in
production shapes). Accumulation stays fp32 in PSUM throughout.

Host-side input prep pre-arranges every tensor into the exact SBUF tile
layout so each DMA reads contiguous per-partition slabs (strided DMA
measured ~2x slower).

Per-core dataflow (all big matmuls N=512):
  qT[e, q] = Wq^T x_q^T              (per super)
  kT-half[e, tok], v-half[tok, e]    -> AllGather within core pair
  scores S^T[k, q] = kT-block^T qT   (accum over 8 e-chunks)
  E = exp((S^T + mask) / 32)         (ACT, fp16 out)
  out[q, e] = (E^T v) / (E^T 1)      (denominator via N=2 ones-matmul)

Softmax max-subtraction is skipped deliberately: logits*scale are
bounded (|s|/32 < ~2.5), so exp is well-conditioned.
"""

import os
import sys

sys.path.insert(0, "/opt/trn_rl_repo")

import numpy as np

import concourse.bass as bass  # noqa: F401
import concourse.tile as tile
from concourse import bacc, mybir
from concourse.bass_utils import run_bass_kernel_spmd

B, T, D = 4, 2048, 1024
P = 128                 # partitions
DC = D // P             # 8 contraction chunks
QSUP = 512              # queries per super
NSUP = 2                # supers per core
NQ = QSUP * NSUP        # 1024 queries per core
SLOT_KT = (16, 8)       # 128-key tiles per super slot (compile-time, all cores)
NKT = sum(SLOT_KT)      # 24
HT = T // 2             # 1024 tokens projected per core (half of the pair)
HCH = HT // P           # 8 token chunks per half
TCH = T // P            # 16 key/value 128-token chunks
SCALE = 1.0 / 32.0      # 1/sqrt(D)
MASK_NEG = -60000.0     # representable in fp16; exp((s+m)/32) == 0

F16 = mybir.dt.float16
F32 = mybir.dt.float32

_CACHE = {}

last_exec_time_ns = None  # set when BASS_KERNEL_TRACE=1


def _build_program():
    nc = bacc.Bacc("TRN2", target_bir_lowering=False, debug=False, num_devices=8)

    xq_d = nc.dram_tensor("xq", [NSUP, P, DC, QSUP], F16, kind="ExternalInput")
    xkv_d = nc.dram_tensor("xkv", [P, DC, HT], F16, kind="ExternalInput")
    wq_d = nc.dram_tensor("wq", [DC, P, DC, P], F16, kind="ExternalInput")
    wk_d = nc.dram_tensor("wk", [P, DC, D], F16, kind="ExternalInput")
    wv_d = nc.dram_tensor("wv", [P, DC, D], F16, kind="ExternalInput")
    msk_d = nc.dram_tensor("msk", [NKT, P, QSUP], F16, kind="ExternalInput")
    out_d = nc.dram_tensor("out", [NQ, D], F32, kind="ExternalOutput")

    with tile.TileContext(nc) as tc:
        with (
            tc.tile_pool(name="wq", bufs=1) as p_wq,
            tc.tile_pool(name="xq", bufs=2) as p_xq,
            tc.tile_pool(name="kt", bufs=1) as p_kt,
            tc.tile_pool(name="v", bufs=1) as p_v,
            tc.tile_pool(name="qt", bufs=2) as p_qt,
            tc.tile_pool(name="misc", bufs=1) as p_misc,
            tc.tile_pool(name="dram", bufs=1, space="DRAM") as p_dram,
            tc.tile_pool(name="ps512", bufs=3, space="PSUM") as ps512,
            tc.tile_pool(name="psav", bufs=2, space="PSUM") as psav,
            tc.tile_pool(name="psd", bufs=1, space="PSUM") as psd,
        ):
            # ---- constants ----
            ones_t = p_misc.tile([P, 2], F16, tag="ones")
            nc.gpsimd.memset(ones_t[:], 1.0)

            # ---- persistent tensors ----
            kt_t = p_kt.tile([P, DC, T], F16)           # k^T  [e, tok]
            v_t = p_v.tile([P, TCH, D], F16)            # v    [tok, e]

            wq_tiles = []

            def q_proj(s):
                """qT[e, q] for super s. DMA order on first call: wq[ec0]
                (gates the first matmul), xq, then remaining wq tiles."""
                first = not wq_tiles

                def load_wq(ec):
                    w = p_wq.tile([P, DC, P], F16, tag=f"wq{ec}")
                    nc.sync.dma_start(w[:], wq_d.ap()[ec])
                    wq_tiles.append(w)

                if first:
                    load_wq(0)
                xq_c = []
                for dc in range(DC):
                    xc = p_xq.tile([P, QSUP], F16, tag=f"xq{dc}")
                    nc.sync.dma_start(xc[:], xq_d.ap()[s][:, dc, :])
                    xq_c.append(xc)
                if first:
                    for ec in range(1, DC):
                        load_wq(ec)

                qt_t = p_qt.tile([P, DC, QSUP], F16, tag="qt")
                for ec in range(DC):
                    acc = ps512.tile([P, QSUP], F32, tag="ps512")
                    for dc in range(DC):
                        nc.tensor.matmul(acc[:], wq_tiles[ec][:, dc, :],
                                         xq_c[dc][:],
                                         start=(dc == 0), stop=(dc == DC - 1))
                    nc.scalar.copy(qt_t[:, ec, :], acc[:])
                return qt_t

            # ---- q projection for super 0 (covers DMA lead-in) ----
            qt_s = [None, None]
            qt_s[0] = q_proj(0)

            with (
                tc.tile_pool(name="wk", bufs=1) as p_wk,
                tc.tile_pool(name="wv", bufs=1) as p_wv,
                tc.tile_pool(name="xkv", bufs=1) as p_xkv,
                tc.tile_pool(name="half", bufs=1) as p_half,
            ):
                # bulk loads in need-order on the sync queue
                wk_t = p_wk.tile([P, DC, D], F16)
                nc.sync.dma_start(wk_t[:], wk_d.ap())
                xkv_t = p_xkv.tile([P, DC, HT], F16)
                nc.sync.dma_start(xkv_t[:], xkv_d.ap())
                wv_t = p_wv.tile([P, DC, D], F16)
                nc.sync.dma_start(wv_t[:], wv_d.ap())

                # ---- P1a: kT for own half -> pair AllGather ----
                ktH = p_half.tile([P, DC, HT], F16, tag="half")
                for kt2 in range(HT // QSUP):
                    for ec in range(DC):
                        acc = ps512.tile([P, QSUP], F32, tag="ps512")
                        for dc in range(DC):
                            nc.tensor.matmul(
                                acc[:], wk_t[:, dc, ec * P:(ec + 1) * P],
                                xkv_t[:, dc, kt2 * QSUP:(kt2 + 1) * QSUP],
                                start=(dc == 0), stop=(dc == DC - 1))
                        nc.scalar.copy(
                            ktH[:, ec, kt2 * QSUP:(kt2 + 1) * QSUP], acc[:])
                kt_in = p_dram.tile([P, DC, HT], F16, tag="kt_in")
                kt_out = p_dram.tile([2, P, DC, HT], F16, tag="kt_out")
                nc.gpsimd.dma_start(kt_in[:], ktH[:])
                nc.gpsimd.collective_compute(
                    "AllGather", mybir.AluOpType.bypass,
                    replica_groups=[[0, 1], [2, 3], [4, 5], [6, 7]],
                    ins=[kt_in.opt()], outs=[kt_out.opt()])
                for h in range(2):
                    nc.gpsimd.dma_start(kt_t[:, :, h * HT:(h + 1) * HT],
                                        kt_out[h])

                # ---- P1b: v for own half -> pair AllGather ----
                vH = p_half.tile([P, HCH, D], F16, tag="half")
                for tk in range(HCH):
                    for eh in range(2):
                        acc = ps512.tile([P, QSUP], F32, tag="ps512")
                        for dc in range(DC):
                            nc.tensor.matmul(
                                acc[:], xkv_t[:, dc, tk * P:(tk + 1) * P],
                                wv_t[:, dc, eh * QSUP:(eh + 1) * QSUP],
                                start=(dc == 0), stop=(dc == DC - 1))
                        nc.vector.tensor_copy(
                            vH[:, tk, eh * QSUP:(eh + 1) * QSUP], acc[:])
                v_in = p_dram.tile([P, HCH, D], F16, tag="v_in")
                v_out = p_dram.tile([2, P, HCH, D], F16, tag="v_out")
                nc.gpsimd.dma_start(v_in[:], vH[:])
                nc.gpsimd.collective_compute(
                    "AllGather", mybir.AluOpType.bypass,
                    replica_groups=[[0, 1], [2, 3], [4, 5], [6, 7]],
                    ins=[v_in.opt()], outs=[v_out.opt()])
                for h in range(2):
                    nc.gpsimd.dma_start(
                        v_t[:, h * HCH:(h + 1) * HCH, :], v_out[h])

            # ---- q projection for super 1 (covers the v AllGather) ----
            qt_s[1] = q_proj(1)

            # ---- P2: per-super scores -> softmax -> att@v ----
            with (
                tc.tile_pool(name="e", bufs=2) as p_e,
                tc.tile_pool(name="msk", bufs=3) as p_m,
                tc.tile_pool(name="sm", bufs=2) as p_sm,
                tc.tile_pool(name="outp", bufs=2) as p_out,
            ):
                kt_base = 0
                for s in range(NSUP):
                    nkt = SLOT_KT[s]
                    qt = qt_s[s]

                    e_t = p_e.tile([P, SLOT_KT[0], QSUP], F16, tag="e")
                    for kt in range(nkt):
                        acc = ps512.tile([P, QSUP], F32, tag="ps512")
                        for ec in range(DC):
                            nc.tensor.matmul(
                                acc[:], kt_t[:, ec, kt * P:(kt + 1) * P],
                                qt[:, ec, :],
                                start=(ec == 0), stop=(ec == DC - 1))
                        m_t = p_m.tile([P, QSUP], F16, tag="m")
                        nc.sync.dma_start(m_t[:], msk_d.ap()[kt_base + kt])
                        sm_t = p_sm.tile([P, QSUP], F32, tag="sm")
                        nc.vector.tensor_add(sm_t[:], acc[:], m_t[:])
                        nc.scalar.activation(e_t[:, kt, :], sm_t[:],
                                             mybir.ActivationFunctionType.Exp,
                                             scale=SCALE)

                    for qs in range(4):
                        o_acc = psav.tile([P, D], F32, tag="av")
                        d_acc = psd.tile([P, 8], F32, tag="d")
                        for kt in range(nkt):
                            lhs = e_t[:, kt, qs * P:(qs + 1) * P]
                            nc.tensor.matmul(o_acc[:, 0:QSUP], lhs,
                                             v_t[:, kt, 0:QSUP],
                                             start=(kt == 0),
                                             stop=(kt == nkt - 1))
                            nc.tensor.matmul(o_acc[:, QSUP:D], lhs,
                                             v_t[:, kt, QSUP:D],
                                             start=(kt == 0),
                                             stop=(kt == nkt - 1))
                            nc.tensor.matmul(d_acc[:, 0:2], lhs, ones_t[:],
                                             start=(kt == 0),
                                             stop=(kt == nkt - 1))
                        dinv = p_misc.tile([P, 1], F32, tag="dinv")
                        nc.vector.reciprocal(dinv[:], d_acc[:, 0:1])
                        o_t = p_out.tile([P, D], F32, tag="o")
                        nc.vector.tensor_scalar_mul(o_t[:], o_acc[:], dinv[:])
                        row = s * QSUP + qs * P
                        nc.sync.dma_start(out_d.ap()[row:row + P, :], o_t[:])
                    kt_base += nkt

    nc.compile()
    return nc


def _prep_weights(Wq16, Wk16, Wv16):
    """Pre-arrange weights into SBUF tile layouts (shared by all cores)."""
    wq = np.ascontiguousarray(
        Wq16.reshape(DC, P, DC, P).transpose(2, 1, 0, 3))   # [ec, p, dc, e]
    wk = np.ascontiguousarray(Wk16.reshape(DC, P, D).swapaxes(0, 1))
    wv = np.ascontiguousarray(Wv16.reshape(DC, P, D).swapaxes(0, 1))
    return wq, wk, wv


def _prep_core_inputs(xT16, wq, wk, wv, b, h):
    """Host-side shard prep for core (batch b, half h)."""
    if h == 0:
        slots = (np.arange(1536, 2048), np.arange(0, 512))
    else:
        slots = (np.arange(1024, 1536), np.arange(512, 1024))
    tq = np.concatenate(slots)

    xTb = xT16[b]                                          # [D, T] fp16
    xq = np.ascontiguousarray(
        xTb[:, tq].reshape(DC, P, NSUP, QSUP).transpose(2, 1, 0, 3))
    xkv = np.ascontiguousarray(
        xTb[:, h * HT:(h + 1) * HT].reshape(DC, P, HT).swapaxes(0, 1))

    masks = np.empty((NKT, P, QSUP), dtype=np.float16)
    base = 0
    for s in range(NSUP):
        kidx = np.arange(SLOT_KT[s] * P).reshape(SLOT_KT[s], P, 1)
        tqs = tq[s * QSUP:(s + 1) * QSUP].reshape(1, 1, QSUP)
        masks[base:base + SLOT_KT[s]] = np.where(
            kidx <= tqs, 0.0, MASK_NEG).astype(np.float16)
        base += SLOT_KT[s]

    return {
        "xq": xq, "xkv": xkv, "wq": wq, "wk": wk, "wv": wv, "msk": masks,
    }, tq


def kernel(x, Wq, Wk, Wv):
    global last_exec_time_ns
    x = np.asarray(x, dtype=np.float32)
    assert x.shape == (B, T, D)

    if "nc" not in _CACHE:
        _CACHE["nc"] = _build_program()
    nc = _CACHE["nc"]

    xT16 = np.ascontiguousarray(
        x.transpose(0, 2, 1)).astype(np.float16)           # [B, D, T]
    wq, wk, wv = _prep_weights(
        np.asarray(Wq, dtype=np.float16),
        np.asarray(Wk, dtype=np.float16),
        np.asarray(Wv, dtype=np.float16))

    in_maps = []
    row_maps = []
    for c in range(8):
        im, tq = _prep_core_inputs(xT16, wq, wk, wv, c // 2, c % 2)
        in_maps.append(im)
        row_maps.append(tq)

    trace = bool(os.environ.get("BASS_KERNEL_TRACE"))
    kw = {}
    if trace:
        kw = {"trace": True, "tmpdir": os.environ.get(
            "BASS_KERNEL_TRACE_DIR", "/tmp/kernel_trace")}
    res = run_bass_kernel_spmd(nc, in_maps, core_ids=list(range(8)), **kw)
    if trace:
        last_exec_time_ns = res.exec_time_ns

    out = np.empty((B, T, D), dtype=np.float32)
    for c in range(8):
        out[c // 2, row_maps[c]] = res.results[c]["out"]
    return out
